# Initial kernel scaffold
#
"""Trainium2 Bass kernel for the 25-bit SNN division iteration.

Math: the reference does a bit-serial two's-complement subtract
R_trial = R - D over 25 LSB-first bit-planes (ripple carry), then
Q = carry_out and R_next = carry_out ? R_trial : R.

Instead of 25 sequential full-adder steps per row, each row's 25 bits are
packed into two exact fp32 integers (lo = bits 0..12, hi = bits 13..24) with
Horner trees, the subtract/borrow/mux runs on the packed values (width = rows,
not rows*bits), and the result is unpacked with fused (mod, is_ge)
tensor_scalar ops.  Everything is exact in fp32 (|values| <= 16383).

Sharding: trivially data-parallel over the batch dim; each of the 8 cores
gets a contiguous block of N/8 rows.
"""

import numpy as np

import concourse.bass as bass
import concourse.mybir as mybir
from concourse.tile import TileContext
from concourse.bass_utils import run_bass_kernel_spmd

N = 2097152
BITS = 25
N_CORES = 8
ROWS = N // N_CORES  # 262144 rows per core
P = 128

F32 = mybir.dt.float32
Alu = mybir.AluOpType


def build(K=256, T=8):
    """Build the per-core Bass module. rows handled = P*K*T."""
    rows = P * K * T
    nc = bass.Bass()

    R_ext = nc.dram_tensor("R", [rows, BITS], F32, kind="ExternalInput")
    D_ext = nc.dram_tensor("D", [rows, BITS], F32, kind="ExternalInput")
    Q_ext = nc.dram_tensor("Q", [rows, 1], F32, kind="ExternalOutput")
    RN_ext = nc.dram_tensor("R_next", [rows, BITS], F32, kind="ExternalOutput")

    # Contiguous per-partition layout: partition p of tile t holds rows
    # [t*P*K + p*K, t*P*K + (p+1)*K), i.e. K*25 consecutive floats.
    Rv = R_ext[:].rearrange("(t p k) b -> t p (k b)", t=T, p=P, k=K)
    Dv = D_ext[:].rearrange("(t p k) b -> t p (k b)", t=T, p=P, k=K)
    RNv = RN_ext[:].rearrange("(t p k) b -> t p (k b)", t=T, p=P, k=K)
    Qv = Q_ext[:].rearrange("(t p k) one -> p t (k one)", t=T, p=P, k=K)

    v = nc.vector

    with TileContext(nc) as tc:
        with (
            tc.tile_pool(name="io", bufs=2) as io,
            tc.tile_pool(name="aux", bufs=2) as aux,
            tc.tile_pool(name="qp", bufs=1) as qp,
        ):
            q_tile = qp.tile([P, T * K], F32)

            for t in range(T):
                r = io.tile([P, K * BITS], F32, tag="r")
                d = io.tile([P, K * BITS], F32, tag="d")
                o = io.tile([P, K * BITS], F32, tag="o")
                nc.sync.dma_start(out=r[:], in_=Rv[t])
                nc.sync.dma_start(out=d[:], in_=Dv[t])

                rb = r[:].rearrange("p (k b) -> p k b", b=BITS)
                db = d[:].rearrange("p (k b) -> p k b", b=BITS)
                ob = o[:].rearrange("p (k b) -> p k b", b=BITS)

                r_lo = aux.tile([P, K], F32, tag="r_lo")
                r_hi = aux.tile([P, K], F32, tag="r_hi")
                d_lo = aux.tile([P, K], F32, tag="d_lo")
                d_hi = aux.tile([P, K], F32, tag="d_hi")

                # Pack trees: lo = sum_{i<13} 2^i b_i, hi = sum_{i>=13} 2^(i-13) b_i
                def pack(dst_lo, dst_hi, src):
                    v.scalar_tensor_tensor(
                        dst_lo[:], src[:, :, 1], 2.0, src[:, :, 0], Alu.mult, Alu.add
                    )
                    for i in range(2, 13):
                        v.scalar_tensor_tensor(
                            dst_lo[:], src[:, :, i], float(2**i), dst_lo[:],
                            Alu.mult, Alu.add,
                        )
                    v.scalar_tensor_tensor(
                        dst_hi[:], src[:, :, 14], 2.0, src[:, :, 13], Alu.mult, Alu.add
                    )
                    for i in range(15, 25):
                        v.scalar_tensor_tensor(
                            dst_hi[:], src[:, :, i], float(2 ** (i - 13)), dst_hi[:],
                            Alu.mult, Alu.add,
                        )

                pack(r_lo, r_hi, rb)
                pack(d_lo, d_hi, db)

                u_lo = aux.tile([P, K], F32, tag="u_lo")
                u_hi = aux.tile([P, K], F32, tag="u_hi")
                ncb_lo = aux.tile([P, K], F32, tag="ncb_lo")
                tp = aux.tile([P, K], F32, tag="tp")
                ncb = aux.tile([P, K], F32, tag="ncb")
                s_lo = aux.tile([P, K], F32, tag="s_lo")
                s_hi = aux.tile([P, K], F32, tag="s_hi")

                v.tensor_sub(u_lo[:], r_lo[:], d_lo[:])
                v.tensor_sub(u_hi[:], r_hi[:], d_hi[:])
                # borrow of low half: u_lo < 0
                v.tensor_scalar(ncb_lo[:], u_lo[:], 0.0, None, Alu.is_lt)
                # tp = u_hi - ncb_lo; carry_out = tp >= 0
                v.scalar_tensor_tensor(tp[:], ncb_lo[:], -1.0, u_hi[:], Alu.mult, Alu.add)
                v.tensor_scalar(
                    q_tile[:, t * K:(t + 1) * K], tp[:], 0.0, None, Alu.is_ge
                )
                v.tensor_scalar(ncb[:], tp[:], 0.0, None, Alu.is_lt)
                # S_lo = u_lo + 8192*ncb_lo ; S_hi = tp + 4096*ncb
                v.scalar_tensor_tensor(s_lo[:], ncb_lo[:], 8192.0, u_lo[:], Alu.mult, Alu.add)
                v.scalar_tensor_tensor(s_hi[:], ncb[:], 4096.0, tp[:], Alu.mult, Alu.add)
                # mux: where borrow (ncb), keep original R
                v.copy_predicated(s_lo[:], ncb[:], r_lo[:])
                v.copy_predicated(s_hi[:], ncb[:], r_hi[:])

                # Unpack: bit_i = (x mod 2^(i+1)) >= 2^i
                for i in range(13):
                    v.tensor_scalar(
                        ob[:, :, i], s_lo[:], float(2 ** (i + 1)), float(2**i),
                        Alu.mod, Alu.is_ge,
                    )
                for i in range(12):
                    v.tensor_scalar(
                        ob[:, :, 13 + i], s_hi[:], float(2 ** (i + 1)), float(2**i),
                        Alu.mod, Alu.is_ge,
                    )

                nc.sync.dma_start(out=RNv[t], in_=o[:])

            nc.sync.dma_start(out=Qv, in_=q_tile[:].rearrange("p (t k) -> p t k", t=T))

    return nc


_nc_full = None


def _get_full_nc():
    global _nc_full
    if _nc_full is None:
        _nc_full = build(K=256, T=8)
    return _nc_full


def _run(R, D, trace=False):
    R = np.ascontiguousarray(np.asarray(R, dtype=np.float32))
    D = np.ascontiguousarray(np.asarray(D, dtype=np.float32))
    assert R.shape == (N, BITS) and D.shape == (N, BITS)
    nc = _get_full_nc()
    in_maps = [
        {"R": R[c * ROWS:(c + 1) * ROWS], "D": D[c * ROWS:(c + 1) * ROWS]}
        for c in range(N_CORES)
    ]
    res = run_bass_kernel_spmd(nc, in_maps, list(range(N_CORES)), trace=trace)
    Q = np.concatenate([res.results[c]["Q"] for c in range(N_CORES)], axis=0)
    RN = np.concatenate([res.results[c]["R_next"] for c in range(N_CORES)], axis=0)
    return (Q, RN), res


def kernel(R, D):
    out, _ = _run(R, D, trace=False)
    return out


# revision 15
# speedup vs baseline: 2.4416x; 2.4416x over previous
"""Trainium2 Bass kernel for the 25-bit SNN division iteration.

Math: the reference does a bit-serial two's-complement subtract
R_trial = R - D over 25 LSB-first bit-planes (ripple carry), then
Q = carry_out and R_next = carry_out ? R_trial : R.

Instead of 25 sequential full-adder steps per row, each row's 25 bits are
packed into two exact fp32 integers (lo = bits 0..12, hi = bits 13..24) with
Horner trees, the subtract/borrow/mux runs on the packed values (width = rows,
not rows*bits), and the result is unpacked with fused (mod, is_ge)
tensor_scalar ops.  Everything is exact in fp32 (|values| <= 16383).

Sharding: trivially data-parallel over the batch dim; each of the 8 cores
gets a contiguous block of N/8 rows.
"""

import numpy as np

import concourse.bass as bass
import concourse.mybir as mybir
from concourse.bacc import Bacc
from concourse.tile import TileContext
from concourse.bass_utils import run_bass_kernel_spmd

N = 2097152
BITS = 25
N_CORES = 8
ROWS = N // N_CORES  # 262144 rows per core
P = 128

F32 = mybir.dt.float32
Alu = mybir.AluOpType


def build(K=256, T=8, reps=1, loop_n=0, internal_io=False):
    """Build the per-core Bass module. rows handled = P*K*T.

    reps>1 (python-unrolled) or loop_n>0 (hardware For_i loop) repeat the
    whole compute loop over the same I/O — used only by the timing harness
    to measure per-iteration HW time via the slope method.

    internal_io=True replaces the big external I/O tensors with on-device
    internal DRAM (contents irrelevant) so timing calls don't pay host
    transfer costs; a tiny passthrough keeps the PJRT plumbing happy.
    """
    rows = P * K * T
    nc = Bacc()

    if internal_io:
        R_ext = nc.dram_tensor("R", [rows, BITS], F32)
        D_ext = nc.dram_tensor("D", [rows, BITS], F32)
        Q_ext = nc.dram_tensor("Q", [rows, 1], F32)
        RN_ext = nc.dram_tensor("R_next", [rows, BITS], F32)
        dum_in = nc.dram_tensor("dummy_in", [P, 8], F32, kind="ExternalInput")
        dum_out = nc.dram_tensor("dummy_out", [P, 8], F32, kind="ExternalOutput")
    else:
        R_ext = nc.dram_tensor("R", [rows, BITS], F32, kind="ExternalInput")
        D_ext = nc.dram_tensor("D", [rows, BITS], F32, kind="ExternalInput")
        Q_ext = nc.dram_tensor("Q", [rows, 1], F32, kind="ExternalOutput")
        RN_ext = nc.dram_tensor("R_next", [rows, BITS], F32, kind="ExternalOutput")

    # Contiguous per-partition layout: partition p of tile t holds rows
    # [t*P*K + p*K, t*P*K + (p+1)*K), i.e. K*25 consecutive floats.
    Rv = R_ext[:].rearrange("(t p k) b -> t p (k b)", t=T, p=P, k=K)
    Dv = D_ext[:].rearrange("(t p k) b -> t p (k b)", t=T, p=P, k=K)
    RNv = RN_ext[:].rearrange("(t p k) b -> t p (k b)", t=T, p=P, k=K)
    Qv = Q_ext[:].rearrange("(t p k) one -> p t (k one)", t=T, p=P, k=K)

    v = nc.vector

    with TileContext(nc) as tc:
        with (
            tc.tile_pool(name="io", bufs=2) as io,
            tc.tile_pool(name="aux", bufs=2) as aux,
            tc.tile_pool(name="qp", bufs=1) as qp,
        ):
            q_tile = qp.tile([P, T * K], F32)

            def rep_body():
              for t in range(T):
                r = io.tile([P, K * BITS], F32, tag="r")
                d = io.tile([P, K * BITS], F32, tag="d")
                o = io.tile([P, K * BITS], F32, tag="o")
                nc.sync.dma_start(out=r[:], in_=Rv[t])
                nc.sync.dma_start(out=d[:], in_=Dv[t])

                rb = r[:].rearrange("p (k b) -> p k b", b=BITS)
                db = d[:].rearrange("p (k b) -> p k b", b=BITS)
                ob = o[:].rearrange("p (k b) -> p k b", b=BITS)

                r_lo = aux.tile([P, K], F32, tag="r_lo")
                r_hi = aux.tile([P, K], F32, tag="r_hi")
                d_lo = aux.tile([P, K], F32, tag="d_lo")
                d_hi = aux.tile([P, K], F32, tag="d_hi")

                # Pack trees: lo = sum_{i<13} 2^i b_i, hi = sum_{i>=13} 2^(i-13) b_i
                def pack(dst_lo, dst_hi, src):
                    v.scalar_tensor_tensor(
                        dst_lo[:], src[:, :, 1], 2.0, src[:, :, 0], Alu.mult, Alu.add
                    )
                    for i in range(2, 13):
                        v.scalar_tensor_tensor(
                            dst_lo[:], src[:, :, i], float(2**i), dst_lo[:],
                            Alu.mult, Alu.add,
                        )
                    v.scalar_tensor_tensor(
                        dst_hi[:], src[:, :, 14], 2.0, src[:, :, 13], Alu.mult, Alu.add
                    )
                    for i in range(15, 25):
                        v.scalar_tensor_tensor(
                            dst_hi[:], src[:, :, i], float(2 ** (i - 13)), dst_hi[:],
                            Alu.mult, Alu.add,
                        )

                pack(r_lo, r_hi, rb)
                pack(d_lo, d_hi, db)

                u_lo = aux.tile([P, K], F32, tag="u_lo")
                u_hi = aux.tile([P, K], F32, tag="u_hi")
                ncb_lo = aux.tile([P, K], F32, tag="ncb_lo")
                tp = aux.tile([P, K], F32, tag="tp")
                ncb = aux.tile([P, K], F32, tag="ncb")
                ncb_i = aux.tile([P, K], mybir.dt.int32, tag="ncb_i")
                s_lo = aux.tile([P, K], F32, tag="s_lo")
                s_hi = aux.tile([P, K], F32, tag="s_hi")

                v.tensor_sub(u_lo[:], r_lo[:], d_lo[:])
                v.tensor_sub(u_hi[:], r_hi[:], d_hi[:])
                # borrow of low half: u_lo < 0
                v.tensor_scalar(ncb_lo[:], u_lo[:], 0.0, None, Alu.is_lt)
                # tp = u_hi - ncb_lo; carry_out = tp >= 0
                v.scalar_tensor_tensor(tp[:], ncb_lo[:], -1.0, u_hi[:], Alu.mult, Alu.add)
                v.tensor_scalar(
                    q_tile[:, t * K:(t + 1) * K], tp[:], 0.0, None, Alu.is_ge
                )
                v.tensor_scalar(ncb[:], tp[:], 0.0, None, Alu.is_lt)
                v.tensor_scalar(ncb_i[:], tp[:], 0.0, None, Alu.is_lt)
                # S_lo = u_lo + 8192*ncb_lo ; S_hi = tp + 4096*ncb
                v.scalar_tensor_tensor(s_lo[:], ncb_lo[:], 8192.0, u_lo[:], Alu.mult, Alu.add)
                v.scalar_tensor_tensor(s_hi[:], ncb[:], 4096.0, tp[:], Alu.mult, Alu.add)
                # mux: where borrow (ncb), keep original R
                v.copy_predicated(s_lo[:], ncb_i[:], r_lo[:])
                v.copy_predicated(s_hi[:], ncb_i[:], r_hi[:])

                # Unpack (shift-subtract cascade, MSB first):
                #   bit_i = (v >= 2^i); v -= 2^i * bit_i
                def unpack(src, nbits, bit_base):
                    for i in range(nbits - 1, 0, -1):
                        dst = ob[:, :, bit_base + i]
                        v.tensor_scalar(dst, src[:], float(2**i), None, Alu.is_ge)
                        v.scalar_tensor_tensor(
                            src[:], dst, -float(2**i), src[:], Alu.mult, Alu.add
                        )
                    v.tensor_copy(ob[:, :, bit_base], src[:])

                unpack(s_lo, 13, 0)
                unpack(s_hi, 12, 13)

                nc.sync.dma_start(out=RNv[t], in_=o[:])

            if loop_n:
                with tc.For_i(0, loop_n, 1):
                    rep_body()
            else:
                for _rep in range(reps):
                    rep_body()

            nc.sync.dma_start(out=Qv, in_=q_tile[:].rearrange("p (t k) -> p t k", t=T))

            if internal_io:
                dt = io.tile([P, 8], F32, tag="dum")
                nc.sync.dma_start(out=dt[:], in_=dum_in[:])
                nc.sync.dma_start(out=dum_out[:], in_=dt[:])

    nc.finalize()
    return nc


_nc_full = None


def _get_full_nc():
    global _nc_full
    if _nc_full is None:
        _nc_full = build(K=256, T=8)
    return _nc_full


def _run(R, D, trace=False):
    R = np.ascontiguousarray(np.asarray(R, dtype=np.float32))
    D = np.ascontiguousarray(np.asarray(D, dtype=np.float32))
    assert R.shape == (N, BITS) and D.shape == (N, BITS)
    nc = _get_full_nc()
    in_maps = [
        {"R": R[c * ROWS:(c + 1) * ROWS], "D": D[c * ROWS:(c + 1) * ROWS]}
        for c in range(N_CORES)
    ]
    res = run_bass_kernel_spmd(nc, in_maps, list(range(N_CORES)), trace=trace)
    Q = np.concatenate([res.results[c]["Q"] for c in range(N_CORES)], axis=0)
    RN = np.concatenate([res.results[c]["R_next"] for c in range(N_CORES)], axis=0)
    return (Q, RN), res


def kernel(R, D):
    out, _ = _run(R, D, trace=False)
    return out
